# revision 6
# baseline (speedup 1.0000x reference)
"""Trainium2 Bass kernel for the gnn_message_passing encoder problem.

kernel(**inputs) takes the FULL inputs and returns the FULL [B, P, R+1] output.

Sharding: 8 cores = 2 batches x 4 object-groups; each core scores 64 padded
(trigger, object) pairs of one document.

v2 pipeline (vs the 56us baseline):
  - The W-sum over span token rows is folded into the host gather (its scale
    cancels in q/qsum), so each core ships only 192 presummed attention rows
    [12 heads x 16 entities, L] in fp8 (393KB instead of 1.57MB).
  - Pair expansion runs as fp8 DoubleRow matmuls (k = 16 entities x 2 head
    parities) at 0.5 cycles/column: one [16,2,128] selector against the
    [16,2,512] attention tile per (supertile, side, L-quarter).
  - Inputs stream over THREE dma queues in parallel (sync + scalar HWDGE,
    gpsimd SWDGE) instead of one.
  - The two head-parity halves of q are folded by a DVE add right after the
    PE transpose, so the context matmul runs at m=64 and the fold2 matmul
    disappears.  qsum rides the context matmul as a fused ones-column.
  - Entity embeddings AND their 12 score chunks are computed in an early
    phase that overlaps the attention stream; the tail only scores the six
    c-chunks in [pair, relation] orientation (no score transpose).
"""

import os
import sys

import numpy as np

for _p in ("/opt/trn_rl_repo", os.path.expanduser("~/.axon_site/_ro/trn_rl_repo")):
    if os.path.isdir(_p) and _p not in sys.path:
        sys.path.insert(0, _p)

import concourse.bass as bass
import concourse.mybir as mybir
import concourse.tile as tile
from concourse import bacc
from concourse.bass_utils import run_bass_kernel_spmd

# Problem dimensions (hardcoded per the harness contract).
B, L, D, H = 2, 2048, 768, 12
E, T, W = 32, 8, 4
R, NN = 57, 20
RN = R + NN            # 77 stacked codebook rows
NE = 16                # entities per core (8 triggers + 8 objects)
NEW = NE * W           # 64 span token rows per head (entity phase)
NP = 64                # pair slots per core (group 0 pads 56 -> 64)
T6 = 6                 # head supertiles (2 heads each)
LQ = 512               # L is processed in 4 slices of 512
DSEQ = D + 8           # seq free dim with ones column at 768 (pad to 776)
NCORES = 8

# Static pair list in the reference's order (s-major).
ALL_PAIRS = [(s, o) for s in range(T) for o in range(E) if s != o]
GROUP_IDX = [[i for i, (_, o) in enumerate(ALL_PAIRS) if o // 8 == g] for g in range(4)]
GROUP_ENTS = [
    list(range(16)),
    list(range(16)),
    list(range(8)) + list(range(16, 24)),
    list(range(8)) + list(range(24, 32)),
]

F32 = mybir.dt.float32
BF16 = mybir.dt.bfloat16
FP8 = mybir.dt.float8e4
import ml_dtypes
NP_BF16 = ml_dtypes.bfloat16
NP_FP8 = ml_dtypes.float8_e4m3

LAST_RESULTS = None  # BassKernelResults of the most recent kernel() call

DR = mybir.MatmulPerfMode.DoubleRow


def _sel_entity(g):
    """Entity mean-pool selectors over span token rows (0.25 scale)."""
    idxs = GROUP_IDX[g]
    ents = GROUP_ENTS[g]
    local = {e: i for i, e in enumerate(ents)}
    sel_s = np.zeros((NEW, NP), np.float32)
    sel_o = np.zeros((NEW, NP), np.float32)
    for j in range(NP):
        s, o = ALL_PAIRS[idxs[j % len(idxs)]]  # pad group 0 by repeating pair 0
        for w in range(W):
            sel_s[local[s] * W + w, j] = 1.0
            sel_o[local[o] * W + w, j] = 1.0
    return sel_s, sel_o


def _sel_pairs(g):
    """DoubleRow pair-expansion selectors [32, side, var, j, 128].

    att partitions hold (supertile, entity) rows in 32-row blocks of two
    supertiles; variant var picks the low/high 16 rows of a block.  Output
    partition m = hh*64 + pair selects (head parity hh, entity of side).
    """
    idxs = GROUP_IDX[g]
    ents = GROUP_ENTS[g]
    local = {e: i for i, e in enumerate(ents)}
    sel = np.zeros((32, 2, 2, 2, 128), np.float32)
    for j in range(NP):
        s, o = ALL_PAIRS[idxs[j % len(idxs)]]
        for side, ent in ((0, s), (1, o)):
            el = local[ent]
            for var in range(2):
                for hh in range(2):
                    sel[var * NE + el, side, var, hh, hh * NP + j] = 1.0
    # replicate at the three 32-partition block offsets used by att_sb
    return np.tile(sel.reshape(1, 32, 2, 2, 2, 128), (3, 1, 1, 1, 1, 1)).reshape(96, 2, 2, 2, 128)


def _build_program(debug=False):
    nc = bacc.Bacc("TRN2")

    att_p = nc.dram_tensor("att_p", [96, 4096], FP8, kind="ExternalInput")
    seq_a = nc.dram_tensor("seq_a", [128, 10 * DSEQ], FP8, kind="ExternalInput")
    seq_b = nc.dram_tensor("seq_b", [128, 6 * DSEQ], FP8, kind="ExternalInput")
    spans = nc.dram_tensor("spans", [NEW, D], BF16, kind="ExternalInput")
    sele = nc.dram_tensor("sele", [NEW, 2 * NP], BF16, kind="ExternalInput")
    selp = nc.dram_tensor("selp", [96, 8 * 128], FP8, kind="ExternalInput")
    rel_t = nc.dram_tensor("rel_t", [128, 18 * RN], BF16, kind="ExternalInput")
    out = nc.dram_tensor("out", [NP, R + 1], F32, kind="ExternalOutput")
    if debug:
        dbg_q = nc.dram_tensor("dbg_q", [128, 16 * NP], F32, kind="ExternalOutput")
        dbg_c2 = nc.dram_tensor("dbg_c2", [NP, DSEQ], F32, kind="ExternalOutput")

    with tile.TileContext(nc) as tc:
        with tc.tile_pool(name="consts", bufs=1) as consts:
            # --- input DMAs, spread over three queues ---
            # sync (SP hwdge): pair selectors, spans, attention, seq 10..15
            selp_sb = consts.tile([96, 2, 2, 2, 128], FP8)
            nc.sync.dma_start(out=selp_sb,
                              in_=selp.rearrange("p (s v j n) -> p s v j n", s=2, v=2, j=2))
            spans_sb = consts.tile([NEW, D], BF16)
            nc.sync.dma_start(out=spans_sb, in_=spans[:, :])
            att_sb = consts.tile([96, 4, 2, LQ], FP8)
            nc.sync.dma_start(out=att_sb,
                              in_=att_p.rearrange("p (q j l) -> p q j l", q=4, j=2))
            seq_sb = consts.tile([128, 16, DSEQ], FP8)
            nc.sync.dma_start(out=seq_sb[:, 10:16, :],
                              in_=seq_b.rearrange("p (c d) -> p c d", c=6))
            # scalar (Act hwdge): entity selectors, seq 0..9
            sele_sb = consts.tile([NEW, 2, NP], BF16)
            nc.scalar.dma_start(out=sele_sb,
                                in_=sele.rearrange("p (s n) -> p s n", s=2))
            nc.scalar.dma_start(out=seq_sb[:, 0:5, :],
                                in_=seq_a.rearrange("p (c d) -> p c d", c=10)[:, 0:5, :])
            nc.scalar.dma_start(out=seq_sb[:, 5:10, :],
                                in_=seq_a.rearrange("p (c d) -> p c d", c=10)[:, 5:10, :])
            # gpsimd (swdge): codebooks
            rel_sb = consts.tile([128, 18, RN], BF16)
            nc.gpsimd.dma_start(out=rel_sb,
                                in_=rel_t.rearrange("p (c n) -> p c n", c=18))

            # bf16 identity for PE transposes, built on device.
            idb_sb = consts.tile([128, 128], BF16)
            nc.gpsimd.memset(idb_sb, 0.0)
            nc.gpsimd.affine_select(
                out=idb_sb, in_=idb_sb,
                compare_op=mybir.AluOpType.not_equal, fill=1.0, base=0,
                pattern=[[-1, 128]], channel_multiplier=1,
            )

            qT2f = consts.tile([128, 16, NP], FP8)    # folded q^T (l-major)
            embsE = consts.tile([128, 12, NP], BF16)  # entity emb chunks
            embsC = consts.tile([128, 6, NP], BF16)   # context emb chunks
            sc_ent = consts.tile([NP, RN], F32)       # entity score partial
            csb = consts.tile([NP, D], BF16)
            sc_f = consts.tile([NP, RN], F32)
            rq = consts.tile([NP, 1], F32)
            fin = consts.tile([NP, R + 1], F32)

            # --- entity phase: embeddings + their 12 score chunks ---
            # Runs while the attention/seq stream is still arriving.
            with tc.tile_pool(name="psE", bufs=1, space="PSUM") as psE:
                e_ps0 = psE.tile([128, 3, 128], F32, tag="e0")
                e_ps1 = psE.tile([128, 3, 128], F32, tag="e1")
                for db in range(6):
                    e_ps = e_ps0 if db < 3 else e_ps1
                    nc.tensor.matmul(
                        out=e_ps[:, db % 3, :],
                        lhsT=spans_sb[:, db * 128:(db + 1) * 128],
                        rhs=sele_sb.rearrange("p s n -> p (s n)"))
                nc.scalar.copy(embsE[:, 0:3, :], e_ps0[:, :, 0:NP])
                nc.vector.tensor_copy(embsE[:, 6:9, :], e_ps0[:, :, NP:128])
                nc.scalar.copy(embsE[:, 3:6, :], e_ps1[:, :, 0:NP])
                nc.vector.tensor_copy(embsE[:, 9:12, :], e_ps1[:, :, NP:128])
                sc_ps = psE.tile([NP, RN], F32, tag="sce")
                for kc in range(12):
                    nc.tensor.matmul(
                        out=sc_ps, lhsT=embsE[:, kc, :], rhs=rel_sb[:, kc, :],
                        start=(kc == 0), stop=(kc == 11))
                nc.vector.tensor_copy(sc_ent, sc_ps)

            # --- main pipeline ---
            # PSUM: psA (2+3 = 5 banks) + psT (1) + psC (2) = 8 banks.
            with tc.tile_pool(name="psT", bufs=1, space="PSUM") as psT, \
                 tc.tile_pool(name="psC", bufs=1, space="PSUM") as psC:
                c2 = psC.tile([NP, DSEQ], F32, tag="c2")

                with tc.tile_pool(name="psA", bufs=2, space="PSUM") as psA, \
                     tc.tile_pool(name="prod", bufs=2) as prod, \
                     tc.tile_pool(name="stg", bufs=3) as stg, \
                     tc.tile_pool(name="q2p", bufs=2) as q2p:
                    for lq in range(4):
                        pmt = prod.tile([128, T6, LQ], BF16, tag="pm")
                        for t in range(T6):
                            a_s = psA.tile([128, LQ], F32, tag="as", bufs=2)
                            a_o = psA.tile([128, LQ], F32, tag="ao", bufs=3)
                            blk, var = divmod(t, 2)
                            rhs = att_sb[blk * 32:(blk + 1) * 32, lq, :, :]
                            lhs_s = selp_sb[blk * 32:(blk + 1) * 32, 0, var, :, :]
                            lhs_o = selp_sb[blk * 32:(blk + 1) * 32, 1, var, :, :]
                            nc.tensor.matmul(out=a_s, lhsT=lhs_s, rhs=rhs, perf_mode=DR)
                            nc.tensor.matmul(out=a_o, lhsT=lhs_o, rhs=rhs, perf_mode=DR)
                            # DVE may read at most one PSUM operand: a_o stays
                            # in PSUM, a_s comes via a staged copy on scalar.
                            ss = stg.tile([128, LQ], BF16, tag="ss")
                            nc.scalar.copy(ss, a_s)
                            nc.vector.tensor_mul(pmt[:, t, :], ss, a_o)
                        # head-sum tree (bf16 SBUF adds; 2x DVE mode)
                        nc.vector.tensor_add(pmt[:, 0, :], pmt[:, 0, :], pmt[:, 1, :])
                        nc.gpsimd.tensor_add(pmt[:, 2, :], pmt[:, 2, :], pmt[:, 3, :])
                        nc.gpsimd.tensor_add(pmt[:, 4, :], pmt[:, 4, :], pmt[:, 5, :])
                        q2t = q2p.tile([128, LQ], BF16, tag="q2")
                        nc.vector.tensor_add(pmt[:, 0, :], pmt[:, 0, :], pmt[:, 2, :])
                        nc.vector.tensor_add(q2t, pmt[:, 0, :], pmt[:, 4, :])
                        # transpose q2 into l-major bf16 PSUM
                        qt_ps = psT.tile([128, 4, 128], BF16, tag="qt", bufs=1)
                        for k in range(4):
                            nc.tensor.matmul(
                                out=qt_ps[:, k, :],
                                lhsT=q2t[:, k * 128:(k + 1) * 128],
                                rhs=idb_sb, is_transpose=True)
                        # stage to SBUF, folding the two head-parity halves
                        ss2 = stg.tile([128, 4, 128], BF16, tag="s2")
                        nc.scalar.copy(ss2, qt_ps)
                        nc.vector.tensor_add(qT2f[:, lq * 4:lq * 4 + 4, :],
                                             ss2[:, :, 0:NP], ss2[:, :, NP:128])
                        # context matmuls for this lq right away (m=64)
                        for cp in (lq * 4, lq * 4 + 2):
                            nc.tensor.matmul(
                                out=c2[:, 0:512], lhsT=qT2f[:, cp:cp + 2, :],
                                rhs=seq_sb[:, cp:cp + 2, 0:512],
                                start=(cp == 0), stop=(cp == 14),
                                perf_mode=DR)
                            nc.tensor.matmul(
                                out=c2[:, 512:DSEQ], lhsT=qT2f[:, cp:cp + 2, :],
                                rhs=seq_sb[:, cp:cp + 2, 512:DSEQ],
                                start=(cp == 0), stop=(cp == 14),
                                perf_mode=DR)

                # tail: normalize c, transpose, score the 6 c-chunks, combine
                nc.vector.reciprocal(rq, c2[:, D:D + 1])
                nc.scalar.mul(csb[:, 0:512], c2[:, 0:512], rq)
                nc.vector.tensor_scalar_mul(csb[:, 512:D], c2[:, 512:D], rq)
                if debug:
                    dc2 = consts.tile([NP, DSEQ], F32)
                    nc.gpsimd.tensor_copy(dc2[:, 0:D], csb)
                    nc.vector.tensor_copy(dc2[:, D:DSEQ], c2[:, D:DSEQ])
                    nc.sync.dma_start(out=dbg_c2[:, :], in_=dc2)

            with tc.tile_pool(name="psF", bufs=1, space="PSUM") as psF:
                cT_ps = psF.tile([128, 6, NP], BF16, tag="cT")
                for db in range(6):
                    nc.tensor.matmul(
                        out=cT_ps[:, db, :],
                        lhsT=csb[:, db * 128:(db + 1) * 128],
                        rhs=idb_sb[0:NP, 0:NP], is_transpose=True)
                nc.scalar.copy(embsC, cT_ps)
                sc2 = psF.tile([NP, RN], F32, tag="sc2")
                for kc in range(6):
                    nc.tensor.matmul(
                        out=sc2, lhsT=embsC[:, kc, :], rhs=rel_sb[:, 12 + kc, :],
                        start=(kc == 0), stop=(kc == 5))
                nc.vector.tensor_add(sc_f, sc2, sc_ent)
                nc.vector.reduce_max(fin[:, 0:1], sc_f[:, R:RN],
                                     axis=mybir.AxisListType.X)
                nc.vector.tensor_copy(fin[:, 1:R + 1], sc_f[:, 0:R])

            nc.sync.dma_start(out=out[:, :], in_=fin)
            if debug:
                dq = consts.tile([128, 16, NP], F32)
                nc.vector.tensor_copy(dq, qT2f)
                nc.sync.dma_start(out=dbg_q.rearrange("p (c n) -> p c n", c=16),
                                  in_=dq)

    return nc


def _in_maps(sequence_output, attention, relation_embeddings, nota_embeddings,
             span_starts):
    sequence_output = np.asarray(sequence_output, np.float32)
    attention = np.asarray(attention, np.float32)
    span_starts = np.asarray(span_starts)
    rel_t = np.ascontiguousarray(
        np.concatenate(
            [np.asarray(relation_embeddings, np.float32),
             np.asarray(nota_embeddings, np.float32)], axis=0
        ).T
    )
    rel_pm = rel_t.astype(NP_BF16).reshape(18, 128, RN).transpose(1, 0, 2)

    in_maps = []
    for c in range(NCORES):
        b, g = divmod(c, 4)
        ents = GROUP_ENTS[g]
        rows = np.concatenate(
            [np.arange(span_starts[b, e], span_starts[b, e] + W) for e in ents]
        )
        # presummed attention rows: [H, NE, L] (W-sum; scale cancels in q/qsum)
        att_rows = attention[b][:, rows, :].reshape(H, NE, W, L).sum(axis=2)
        # [96, 4, 2, 512]: partition hp*16+e, free (lq, head parity, l)
        att_q = att_rows.reshape(T6, 2, NE, 4, LQ).transpose(0, 2, 3, 1, 4)
        seq_pm = np.zeros((128, 16, DSEQ), NP_FP8)
        seq_pm[:, :, 0:D] = sequence_output[b].astype(NP_FP8).reshape(16, 128, D).transpose(1, 0, 2)
        seq_pm[:, :, D] = 1.0
        sel_s, sel_o = _sel_entity(g)
        sele_h = np.concatenate([sel_s * 0.25, sel_o * 0.25], axis=1)  # [64, 128]
        selp_h = _sel_pairs(g)  # [16, 2, 2, 128]
        in_maps.append({
            "att_p": np.ascontiguousarray(att_q.astype(NP_FP8).reshape(96, 4096)),
            "seq_a": np.ascontiguousarray(seq_pm[:, 0:10, :].reshape(128, 10 * DSEQ)),
            "seq_b": np.ascontiguousarray(seq_pm[:, 10:16, :].reshape(128, 6 * DSEQ)),
            "spans": np.ascontiguousarray(sequence_output[b][rows].astype(NP_BF16)),
            "sele": np.ascontiguousarray(sele_h.astype(NP_BF16)),
            "selp": np.ascontiguousarray(selp_h.astype(NP_FP8).reshape(96, 8 * 128)),
            "rel_t": np.ascontiguousarray(rel_pm.reshape(128, 18 * RN)),
        })
    return in_maps


def kernel(sequence_output, attention, relation_embeddings, nota_embeddings,
           span_starts):
    global LAST_RESULTS
    in_maps = _in_maps(sequence_output, attention, relation_embeddings,
                       nota_embeddings, span_starts)
    nc = _build_program()
    nc.finalize()  # Bacc legalization (wait splitting, reg alloc)
    LAST_RESULTS = run_bass_kernel_spmd(nc, in_maps, core_ids=list(range(NCORES)))

    out = np.zeros((B, len(ALL_PAIRS), R + 1), np.float32)
    for c in range(NCORES):
        b, g = divmod(c, 4)
        idxs = GROUP_IDX[g]
        out[b, idxs, :] = LAST_RESULTS.results[c]["out"][: len(idxs)]
    return out
